# revision 12
# baseline (speedup 1.0000x reference)
"""Causal self-attention (B=4, T=2048, C=1024, H=16, Dh=64) on 8 NeuronCores.

Fused single-pass kernel, core = g*4 + b (b = batch, g = head-group of 8
heads; the two head-group proj partials per batch are summed on the host).
All matmul inputs bf16 (fp32 PSUM accumulate). The PE instruction stream is
hand-interleaved so the PE never micro-idles — the HAM activity monitor then
keeps the clock at 2.4 GHz and the row-tiled S-pairs run concurrently:

  prologue: qkv for x-chunk 0 (inputs DMAed across the 3 queues)
  for qb (512-wide q block), for j (head pair):
    S-pair bursts (64-row-tiled matmuls, 2 heads concurrent)
    -> exp on ACT (ACT does nothing else) + diag-mask mult on GpSimd/DVE
    -> PV bursts (128-mode, lag 2 groups) accumulating out^T plus a
       ones-column denominator row, with qkv/proj units for other chunks
       popped from a filler queue between groups to keep the PE dense
       (queue budgets reserve work for the exp-bound final region);
    normalize per (j, qb): denom rows gathered via DMA into a [64,64]
    tile (partition-parallel reciprocal), DRAM-bounce partition
    broadcast, DVE multiply straight into attnT. The last unit instead
    uses ACT exp(-ln x) + a K=1 PE broadcast to avoid the DMA latency
    chain at the tail.
  proj units for a qb join the filler queue once all 4 pairs are normed;
  leftovers drain before the final unit's exp-blocked PV tail.
"""

from collections import deque

import numpy as np

import concourse.bass as bass
import concourse.mybir as mybir
import concourse.tile as tile
from concourse.bass_utils import run_bass_kernel_spmd

FP = mybir.dt.float32
BF = mybir.dt.bfloat16
AF = mybir.ActivationFunctionType

T = 2048
C = 1024
DH = 64
G = 512
N_CORES = 8

_NO_SPLIT_OPCODES = ()


def _split_multi_waits(nc, max_waits=1):
    """This walrus build rejects >1 sync wait per engine instruction.
    Hoist extra waits onto single-wait NOPs inserted just before the
    instruction on the same engine (engine streams execute in bb order)."""
    fixes = {}
    bbs = [bb for fn in nc.m.functions for bb in fn.blocks]
    for bb in bbs:
        for inst in list(bb.instructions):
            si = inst.sync_info
            if si is None or not si.on_wait:
                continue
            waits = list(si.on_wait)
            if len(waits) <= max_waits:
                continue
            op = str(inst.opcode)
            if any(t in op for t in _NO_SPLIT_OPCODES):
                continue
            if inst.engine not in nc.engines:
                continue
            extra, keep = waits[:-max_waits], waits[-max_waits:]
            nops = []
            for w in extra:
                nop = nc.engines[inst.engine].nop(nofuse=True).ins
                nop.sync_info = mybir.SyncInfo(on_wait=[w], on_update=[])
                nops.append(nop)
            si.on_wait = keep
            fixes[inst.name] = nops
    if not fixes:
        return
    appended = {n.name for nops in fixes.values() for n in nops}
    for bb in bbs:
        new_insts = []
        for inst in bb.instructions:
            if inst.name in appended:
                continue
            new_insts.extend(fixes.get(inst.name, ()))
            new_insts.append(inst)
        bb.instructions = new_insts


def _build_nc():
    nc = bass.Bass("TRN2", target_bir_lowering=False, debug=False)
    xT = nc.dram_tensor("xT", [C, T], BF, kind="ExternalInput")
    wqk = nc.dram_tensor("wqk", [C, 2 * G], BF, kind="ExternalInput")
    wv = nc.dram_tensor("wv", [C, G], BF, kind="ExternalInput")
    wproj = nc.dram_tensor("wproj", [G, C], BF, kind="ExternalInput")
    maskones = nc.dram_tensor("maskones", [128, 132], BF, kind="ExternalInput")
    out = nc.dram_tensor("out", [T, C], BF, kind="ExternalOutput")

    with tile.TileContext(nc) as tc:
        with (
            tc.tile_pool(name="persist", bufs=1) as persist,
            tc.tile_pool(name="ptp", bufs=8) as ptpool,
            tc.tile_pool(name="stgp", bufs=4) as stgpool,
            tc.tile_pool(name="colp", bufs=2) as colpool,
            tc.tile_pool(name="bcp", bufs=4) as bcpool,
            tc.tile_pool(name="pop", bufs=4) as popool,
            tc.tile_pool(name="dscr", bufs=4, space="DRAM") as dpool,
            tc.tile_pool(name="mm128", bufs=2, space="PSUM") as mmpool,
            tc.tile_pool(name="spp", bufs=2, space="PSUM") as sppool,
            tc.tile_pool(name="oTe", bufs=1, space="PSUM") as oepool,
            tc.tile_pool(name="oTo", bufs=1, space="PSUM") as oopool,
        ):
            # ---------------- input DMAs, spread across queues ----------------
            def x_ap(c):
                return xT[:, c * 512 : (c + 1) * 512].rearrange(
                    "(k p) t -> p k t", p=128
                )

            def wqk_ap(mh):
                return wqk[:, mh * 512 : (mh + 1) * 512].rearrange(
                    "(k p) c -> p k c", p=128
                )

            # first-unit data (x0 + wqkt0, 2MB) balanced ~683KB per queue
            x0 = persist.tile([128, 8, 512], BF, name="x0")
            nc.sync.dma_start(out=x0[:, 0:4, :], in_=x_ap(0)[:, 0:4, :])
            nc.gpsimd.dma_start(out=x0[:, 4:8, :], in_=x_ap(0)[:, 4:8, :])
            wqkt = [persist.tile([128, 8, 512], BF, name="wqkt0")]
            nc.scalar.dma_start(out=wqkt[0][:, 0:6, :], in_=wqk_ap(0)[:, 0:6, :])
            nc.sync.dma_start(out=wqkt[0][:, 6:8, :], in_=wqk_ap(0)[:, 6:8, :])
            wqkt.append(persist.tile([128, 8, 512], BF, name="wqkt1"))
            nc.scalar.dma_start(out=wqkt[1][:, 0:4, :], in_=wqk_ap(1)[:, 0:4, :])
            nc.sync.dma_start(out=wqkt[1][:, 4:8, :], in_=wqk_ap(1)[:, 4:8, :])
            wqk_sb = {(k, mh): wqkt[mh][:, k, :] for k in range(8) for mh in range(2)}

            mask_sb = persist.tile([128, 132], BF)
            nc.gpsimd.dma_start(out=mask_sb[:], in_=maskones[:])

            wvt = persist.tile([128, 8, 512], BF, name="wvt")
            nc.gpsimd.dma_start(
                out=wvt[:], in_=wv[:].rearrange("(k p) c -> p k c", p=128)
            )
            wv_sb = [wvt[:, k, :] for k in range(8)]

            x_sb = [x0]
            for c, eng in ((1, nc.sync), (2, nc.scalar), (3, nc.gpsimd)):
                xt = persist.tile([128, 8, 512], BF, name=f"x{c}")
                eng.dma_start(out=xt[:], in_=x_ap(c))
                x_sb.append(xt)

            wproj_sb = persist.tile([128, 4, C], BF)
            nc.scalar.dma_start(
                out=wproj_sb[:], in_=wproj[:].rearrange("(k p) c -> p k c", p=128)
            )

            qT_sb = persist.tile([128, 4, T], BF, tag="qT")
            kT_sb = persist.tile([128, 4, T], BF, tag="kT")
            v_sb = persist.tile([128, 16, 8, 66], BF, tag="v")
            attnT_sb = persist.tile([128, 4, T], BF, tag="attnT")

            # ones columns for the softmax denominator
            nc.vector.tensor_copy(
                v_sb[:, :, :, 64],
                mask_sb[:, 128:129].to_broadcast((128, 16 * 8)),
            )

            # ---------------- work unit emitters ----------------
            def emit_qk(c, m):
                ps = mmpool.tile([128, 512], FP, tag="mm", name=f"qk{c}_{m}")
                for k in range(8):
                    nc.tensor.matmul(
                        ps[:],
                        wqk_sb[(k, m // 4)][:, (m % 4) * 128 : (m % 4 + 1) * 128],
                        x_sb[c][:, k, :],
                        start=(k == 0),
                        stop=(k == 7),
                    )
                dst = qT_sb if m < 4 else kT_sb
                nc.vector.tensor_copy(
                    dst[:, m % 4, c * 512 : (c + 1) * 512], ps[:]
                )

            def emit_v(tt):
                ps = mmpool.tile([128, 512], FP, tag="mm", name=f"v{tt}")
                for k in range(8):
                    nc.tensor.matmul(
                        ps[:],
                        x_sb[tt // 4][:, k, (tt % 4) * 128 : (tt % 4 + 1) * 128],
                        wv_sb[k],
                        start=(k == 0),
                        stop=(k == 7),
                    )
                nc.vector.tensor_copy(
                    v_sb[:, tt, :, 0:64], ps[:].rearrange("p (h d) -> p h d", h=8)
                )

            def emit_proj(tt, n):
                ps = mmpool.tile([128, 512], FP, tag="mm", name=f"pj{tt}_{n}")
                for k in range(4):
                    nc.tensor.matmul(
                        ps[:],
                        attnT_sb[:, k, tt * 128 : (tt + 1) * 128],
                        wproj_sb[:, k, n * 512 : (n + 1) * 512],
                        start=(k == 0),
                        stop=(k == 3),
                    )
                po = popool.tile([128, 512], BF, tag="po", name=f"po{tt}_{n}")
                nc.vector.tensor_copy(po[:], ps[:])
                if tt >= 12:
                    eng = (nc.sync, nc.gpsimd, nc.scalar)[(2 * tt + n) % 3]
                else:
                    eng = (nc.sync, nc.gpsimd)[(2 * tt + n) % 2]
                eng.dma_start(
                    out=out[tt * 128 : (tt + 1) * 128, n * 512 : (n + 1) * 512],
                    in_=po[:],
                )

            # ---------------- main fused schedule ----------------
            fillers = deque()  # entries: (req_qb, fn); req_qb = earliest qb
            budget = [0]       # that must have this unit emitted already
            slot = [0]
            ration = [False]

            def pop_filler(force=False):
                slot[0] += 1
                if ration[0] and slot[0] % 2 != 1 and not force:
                    return
                if fillers and budget[0] > 0:
                    budget[0] -= 1
                    fillers.popleft()[1]()

            def drain_required(qb):
                rest = deque()
                for req, fn in fillers:
                    if req <= qb:
                        fn()
                    else:
                        rest.append((req, fn))
                fillers.clear()
                fillers.extend(rest)

            # prologue: chunk 0 qkv (q pairs first: wqk mh0 lands first)
            for m in (0, 1, 2, 3, 4, 5, 6, 7):
                emit_qk(0, m)
            for mt in range(4):
                emit_v(mt)
            for c in range(1, 4):
                for m in (0, 4, 1, 5, 2, 6, 3, 7):
                    fillers.append((c, lambda c=c, m=m: emit_qk(c, m)))
                for mt in range(4):
                    fillers.append((c, lambda c=c, mt=mt: emit_v(4 * c + mt)))

            for qb in range(4):
                nk = 4 * qb + 4
                q0 = qb * 512
                drain_required(qb)
                budget[0] = (99, 12, 12, 99)[qb]
                slot[0] = 0
                ration[0] = qb == 3
                for j in range(4):
                    oT_e = oepool.tile([65, 512], FP, tag="oe", name=f"oe{j}_{qb}")
                    oT_o = oopool.tile([65, 512], FP, tag="oo", name=f"oo{j}_{qb}")
                    pts = {}

                    def emit_pv(tk, last):
                        m = tk - 4 * qb
                        c0 = 0 if m < 1 else 128 * m
                        pt = pts.pop(tk)
                        nc.tensor.matmul(
                            oT_e[:, c0:512],
                            v_sb[:, tk, 2 * j, 0:65],
                            pt[:, c0:512],
                            start=(tk == 0),
                            stop=last,
                        )
                        nc.tensor.matmul(
                            oT_o[:, c0:512],
                            v_sb[:, tk, 2 * j + 1, 0:65],
                            pt[:, 512 + c0 : 1024],
                            start=(tk == 0),
                            stop=last,
                        )

                    for g in range(nk // 2):
                        for tk in (2 * g, 2 * g + 1):
                            m = tk - 4 * qb
                            c0 = 0 if m < 1 else 128 * m
                            k0 = tk * 128
                            sp = sppool.tile(
                                [128, 1024], FP, tag="s", name=f"s{j}{qb}{tk}"
                            )
                            nc.tensor.matmul(
                                sp[:, c0:512],
                                kT_sb[0:64, j, k0 : k0 + 128],
                                qT_sb[0:64, j, q0 + c0 : q0 + 512],
                                start=True,
                                stop=True,
                                tile_position=(0, 0),
                            )
                            nc.tensor.matmul(
                                sp[:, 512 + c0 : 1024],
                                kT_sb[64:128, j, k0 : k0 + 128],
                                qT_sb[64:128, j, q0 + c0 : q0 + 512],
                                start=True,
                                stop=True,
                                tile_position=(64, 0),
                            )
                            pt = ptpool.tile(
                                [128, 1024], BF, tag="pt", name=f"pt{j}{qb}{tk}"
                            )
                            if c0 == 0:
                                nc.scalar.activation(pt[:], sp[:], AF.Exp, scale=0.125)
                            else:
                                sp3 = sp[:].rearrange("p (h x) -> p h x", h=2)[
                                    :, :, c0:512
                                ]
                                pt3 = pt[:].rearrange("p (h x) -> p h x", h=2)[
                                    :, :, c0:512
                                ]
                                nc.scalar.activation(pt3, sp3, AF.Exp, scale=0.125)
                            if m >= 0:  # mask the 128-wide diagonal strip
                                for hh in (0, 1):
                                    o0 = hh * 512 + c0
                                    eng = nc.gpsimd if hh == 0 else nc.vector
                                    eng.tensor_mul(
                                        pt[:, o0 : o0 + 128],
                                        pt[:, o0 : o0 + 128],
                                        mask_sb[:, 0:128],
                                    )
                            pts[tk] = pt
                        if g % 2 == 1 and g >= 3:
                            for tk in range(4 * (g // 2) - 4, 4 * (g // 2)):
                                emit_pv(tk, False)
                        if qb == 3 and j == 3:
                            if g >= 4:
                                pop_filler()
                                pop_filler()
                        else:
                            pop_filler()
                    if qb == 3 and j == 3:
                        for _ in range(12):
                            pop_filler(force=True)
                    for tk in sorted(pts):
                        emit_pv(tk, tk == nk - 1)
                    # evict + normalize
                    stg_e = stgpool.tile([65, 512], FP, tag="st", name=f"se{j}{qb}")
                    nc.vector.tensor_copy(stg_e[:], oT_e[:])
                    stg_o = stgpool.tile([65, 512], FP, tag="st", name=f"so{j}{qb}")
                    nc.vector.tensor_copy(stg_o[:], oT_o[:])
                    if qb == 3 and j == 3:
                        # final unit: low-latency path — ACT reciprocal
                        # (exp(-ln x)) + PE partition-broadcast, no DMA legs
                        rec = colpool.tile([1, 1024], BF, tag="recf", name="recf")
                        for hh, oT in ((0, oT_e), (1, oT_o)):
                            r = rec[0:1, hh * 512 : hh * 512 + 512]
                            nc.scalar.activation(r, oT[64:65, :], AF.Ln)
                            nc.scalar.activation(r, r, AF.Exp, scale=-1.0)
                        for hh, stg in ((0, stg_e), (1, stg_o)):
                            bcp_ = sppool.tile(
                                [128, 1024], FP, tag="s", name=f"bps{hh}"
                            )
                            nc.tensor.matmul(
                                bcp_[0:64, 0:512],
                                mask_sb[0:1, 0:64],
                                rec[0:1, hh * 512 : hh * 512 + 512],
                                start=True,
                                stop=True,
                            )
                            nc.vector.tensor_mul(
                                attnT_sb[hh * 64 : (hh + 1) * 64, j, q0 : q0 + 512],
                                stg[0:64, :],
                                bcp_[0:64, 0:512],
                            )
                    else:
                        col = colpool.tile([64, 64], FP, tag="col", name=f"cl{j}{qb}")
                        nc.vector.memset(col[:], 1.0)
                        nc.sync.dma_start(out=col[0:8, :], in_=stg_e[64:65, :])
                        nc.sync.dma_start(out=col[8:16, :], in_=stg_o[64:65, :])
                        rec = colpool.tile([64, 64], FP, tag="rec", name=f"rc{j}{qb}")
                        nc.vector.reciprocal(rec[0:32, :], col[0:32, :])
                        dscr = dpool.tile([2, 512], FP, tag="d", name=f"dc{j}{qb}")
                        nc.sync.dma_start(out=dscr[:], in_=rec[0:16, :])
                        for hh, stg in ((0, stg_e), (1, stg_o)):
                            bc = bcpool.tile(
                                [64, 512], FP, tag="bc", name=f"bc{j}{qb}{hh}"
                            )
                            nc.gpsimd.dma_start(
                                out=bc[:],
                                in_=dscr[hh : hh + 1, :].to_broadcast((64, 512)),
                            )
                            nc.vector.tensor_mul(
                                attnT_sb[hh * 64 : (hh + 1) * 64, j, q0 : q0 + 512],
                                stg[0:64, :],
                                bc[:],
                            )
                # proj for this qb becomes available filler work
                for tt in range(4 * qb, 4 * qb + 4):
                    for n in range(2):
                        fillers.append((9, lambda tt=tt, n=n: emit_proj(tt, n)))

            while fillers:
                fillers.popleft()[1]()

    _split_multi_waits(nc)
    return nc


_NC_CACHE = None


def _get_nc():
    global _NC_CACHE
    if _NC_CACHE is None:
        _NC_CACHE = _build_nc()
    return _NC_CACHE


def _maskones():
    i = np.arange(128)[:, None]
    c = np.arange(132)[None, :]
    m = (c >= i).astype(np.float32)
    m[:, 128] = 1.0
    m[:, 129:] = 0.0
    return m


def _bf16(a):
    import ml_dtypes

    return np.asarray(a, dtype=np.float32).astype(ml_dtypes.bfloat16)


def _in_maps(x, w_qkv, w_proj):
    maskones = _bf16(_maskones())
    maps = []
    for core in range(N_CORES):
        b, g = core % 4, core // 4
        maps.append(
            {
                "xT": _bf16(np.ascontiguousarray(x[b].T)),
                "wqk": _bf16(
                    np.ascontiguousarray(
                        np.concatenate(
                            [
                                w_qkv[:, g * G : (g + 1) * G],
                                w_qkv[:, C + g * G : C + (g + 1) * G],
                            ],
                            axis=1,
                        )
                    )
                ),
                "wv": _bf16(
                    np.ascontiguousarray(w_qkv[:, 2 * C + g * G : 2 * C + (g + 1) * G])
                ),
                "wproj": _bf16(np.ascontiguousarray(w_proj[g * G : (g + 1) * G, :])),
                "maskones": maskones,
            }
        )
    return maps


def _run(x, w_qkv, w_proj, **spmd_kwargs):
    nc = _get_nc()
    res = run_bass_kernel_spmd(
        nc, _in_maps(x, w_qkv, w_proj), core_ids=list(range(N_CORES)), **spmd_kwargs
    )
    outs = res.results
    full = np.empty((4, T, C), np.float32)
    for b in range(4):
        full[b] = np.asarray(outs[b]["out"], np.float32) + np.asarray(
            outs[4 + b]["out"], np.float32
        )
    return full, res


def kernel(x, w_qkv, w_proj):
    full, _ = _run(
        np.asarray(x, np.float32),
        np.asarray(w_qkv, np.float32),
        np.asarray(w_proj, np.float32),
    )
    return full
